# revision 8
# baseline (speedup 1.0000x reference)
"""Trainium2 Bass kernel for Mistral-style sliding-window GQA attention.

Problem: B=2, T=2048, C=2048, 32 q heads / 8 kv heads, head_dim=64,
sliding causal window 1024, RoPE, fp32.

Sharding (sequence-parallel, no cross-core communication):
  core c in 0..7 handles batch b=c//4 and contiguous 512-row chunk k=c%4.
  Each core computes q for its 512 rows, k/v for its rows plus a 1024-row
  halo (zero-padded before t=0), full attention for its rows over all 32
  heads, and the output projection for its rows.  Host gathers by
  concatenation only.

Schedule (v2 — keeps the PE continuously fed):
  - KV projection per 512-col third: all V matmuls (ci-outer, st-inner)
    first, then K matmuls m-outer/ci-inner.  Rope + vext copies for third
    t overlap the matmuls of third t+1, so the PSUM WAR never stalls PE.
  - The xkv third covering the core's own 512 tokens doubles as the
    Q-projection moving operand (resident tiles, no re-DMA).
  - Q projection is emitted as 16 per-ci rounds (4 matmuls each)
    interleaved between attention units of the previous head-group pair,
    filling the PE bubbles left by the ACT-bound exp chain.
  - qT is a rotating 2-buffer: attention phase p only reads the tile
    written by q sweep p.
  - Output projection: oc0's k-contraction rounds interleave into the
    last attention phase (PSUM banks freed by the finished q sweeps);
    oc1..3 run tt-outer/k-inner so the next oc never waits on staging.
  - scores in S^T=[key, query] layout with the 4 q heads of a kv group
    packed in the moving operand (N=512, keeps float32r at 1 cyc/row);
    PV uses V_ext (64 dims + validity column) as stationary so row 64
    accumulates the softmax denominator; no max-subtraction (inputs are
    N(0,1)-scaled, exp stays in fp32 range).
"""

import numpy as np

import concourse.bass as bass
import concourse.mybir as mybir
import concourse.tile as tile
from concourse import bacc
from concourse.bass_utils import run_bass_kernel_spmd

B, T, C = 2, 2048, 2048
NH, NKV, D = 32, 8, 64
REP = NH // NKV
WIN = 1024
CH = 512          # q rows per core
KVR = CH + WIN    # kv rows per core (with halo)
NCORE = 8
DT = mybir.dt.float32
F32R = mybir.dt.float32r
SCALE = 1.0 / np.sqrt(np.float32(D))
ROPE_BASE = 10000.0

FD = T // 128     # 16 contraction tiles of the model dim
NQT = CH // 128   # 4 q tiles per chunk
NKB = KVR // 128  # 12 kv blocks per core
NWB = 9           # kv blocks in the window of one q tile
VW = 65           # v_ext width per kv block (64 dims + validity column)
VP = NKB * VW     # per-head v_ext pitch (780)


def _rope_write(nc, pool, out_ap, ps, cosw, ssinw, n, swap_engine=None):
    """out = ps*cos + rot_half(ps)*sin on a [128, n] 2-head-packed tile.

    ssinw rows carry the rotate-half signs (rows 0-31/64-95 negated) and any
    folded scale; cosw carries the same scale.  out_ap is either one [128, n]
    AP or a list of two ([64, n] AP) halves receiving rows 0:64 / 64:128.

    If swap_engine is given (an idle PSUM-capable engine, e.g. nc.scalar),
    the rotate-half shuffle is materialized there with 4 quarter copies and
    the vector engine does only 3 full-width ops; otherwise the vector
    engine does 4 quarter multiplies + 2 full ops.
    """
    if swap_engine is not None:
        sw = pool.tile([128, n], DT, tag="rope_sw", name="rope_sw")
        swap_engine.copy(sw[0:32, :], ps[32:64, :])
        swap_engine.copy(sw[32:64, :], ps[0:32, :])
        swap_engine.copy(sw[64:96, :], ps[96:128, :])
        swap_engine.copy(sw[96:128, :], ps[64:96, :])
        t2 = pool.tile([128, n], DT, tag="rope_t2", name="rope_t2")
        nc.vector.tensor_mul(t2[:], sw[:], ssinw[:])
    else:
        t2 = pool.tile([128, n], DT, tag="rope_t2", name="rope_t2")
        nc.vector.tensor_mul(t2[0:32, :], ps[32:64, :], ssinw[0:32, :])
        nc.vector.tensor_mul(t2[32:64, :], ps[0:32, :], ssinw[32:64, :])
        nc.vector.tensor_mul(t2[64:96, :], ps[96:128, :], ssinw[64:96, :])
        nc.vector.tensor_mul(t2[96:128, :], ps[64:96, :], ssinw[96:128, :])
    t1 = pool.tile([128, n], DT, tag="rope_t1", name="rope_t1")
    nc.vector.tensor_mul(t1[:], ps[:], cosw[:])
    if isinstance(out_ap, list):
        for i, half in enumerate(out_ap):
            nc.gpsimd.tensor_add(half, t1[64 * i:64 * (i + 1), :],
                                 t2[64 * i:64 * (i + 1), :])
    else:
        nc.gpsimd.tensor_add(out_ap, t1[:], t2[:])


def build_program():
    nc = bacc.Bacc("TRN2", target_bir_lowering=False, debug=False,
                   num_devices=NCORE)

    xkv_d = nc.dram_tensor("xkv", [C, KVR], F32R, kind="ExternalInput")
    wq_d = nc.dram_tensor("wq", [C, NH * D], F32R, kind="ExternalInput")
    wk_d = nc.dram_tensor("wk", [C, NKV * D], F32R, kind="ExternalInput")
    wv_d = nc.dram_tensor("wv", [C, NKV * D], F32R, kind="ExternalInput")
    wo_d = nc.dram_tensor("wo", [NH * D, C], F32R, kind="ExternalInput")
    rqc_d = nc.dram_tensor("rope_q_cos", [128, CH], DT, kind="ExternalInput")
    rqs_d = nc.dram_tensor("rope_q_sin", [128, CH], DT, kind="ExternalInput")
    rkc_d = nc.dram_tensor("rope_k_cos", [128, KVR], DT, kind="ExternalInput")
    rks_d = nc.dram_tensor("rope_k_sin", [128, KVR], DT, kind="ExternalInput")
    kvv_d = nc.dram_tensor("kvvalid", [128, NKB], F32R, kind="ExternalInput")
    mw_d = nc.dram_tensor("mask_win8", [128, 512], F32R, kind="ExternalInput")
    mc_d = nc.dram_tensor("mask_causal8", [128, 512], F32R,
                          kind="ExternalInput")
    out_d = nc.dram_tensor("out", [CH, C], DT, kind="ExternalOutput")

    NTH = 3          # x-column thirds
    QW = KVR // NTH  # 512 columns per third

    with tile.TileContext(nc) as tc:
        with (
            tc.tile_pool(name="const", bufs=1) as cpool,
            tc.tile_pool(name="qT", bufs=2) as qT_pool,
            tc.tile_pool(name="kT", bufs=1) as kT_pool,
            tc.tile_pool(name="vext", bufs=1) as v_pool,
            tc.tile_pool(name="aT", bufs=1) as aT_pool,
        ):
            # ---- constants (small, persistent) ----
            mask_win = cpool.tile([128, 512], F32R, tag="mw", name="mask_win")
            nc.gpsimd.dma_start(mask_win[:], mw_d[:, :])
            mask_causal = cpool.tile([128, 512], F32R, tag="mc",
                                     name="mask_causal")
            nc.gpsimd.dma_start(mask_causal[:], mc_d[:, :])
            kvv = cpool.tile([128, NKB], F32R, tag="kvv", name="kvv")
            nc.gpsimd.dma_start(kvv[:], kvv_d[:, :])

            # kT: [d, t] packed 2 kv heads per tile.
            kT = [kT_pool.tile([128, KVR], F32R, tag=f"kT{i}", name=f"kT{i}")
                  for i in range(NKV // 2)]
            # vext: one tile, head kvh at pitch VP; per block 64 dims+validity
            vext = v_pool.tile([128, NKV * VP], F32R, tag="vext", name="vext")
            # aT: attention output, [d, t], 2 heads per tile.
            aT = [aT_pool.tile([128, CH], F32R, tag=f"aT{i}", name=f"aT{i}")
                  for i in range(NH // 2)]
            # qT[p]: rotating per-sweep tile, rows 0:64 = group 2p (its 4
            # heads side by side), rows 64:128 = group 2p+1.
            qT = [None] * 4

            with tc.tile_pool(name="xq_res", bufs=1) as xq_pool:
                # resident x tiles for the core's own 512 tokens: the third
                # qu==2 of xkv, reused as the Q-projection moving operand.
                xq = [xq_pool.tile([128, CH], F32R, tag=f"xq{ci}",
                                   name=f"xq{ci}") for ci in range(FD)]
                for ci in range(FD):
                    nc.gpsimd.dma_start(
                        xq[ci][:],
                        xkv_d[128 * ci:128 * (ci + 1), WIN:WIN + CH])

                # ================= KV projection =================
                with (
                    tc.tile_pool(name="rk_tab", bufs=1) as rk_pool,
                    tc.tile_pool(name="wkm_s", bufs=8) as wkm_pool,
                    tc.tile_pool(name="wv_s", bufs=4) as wv_pool,
                    tc.tile_pool(name="xkv_s", bufs=18) as xkv_pool,
                    tc.tile_pool(name="rope_tmp", bufs=2) as rtmp,
                    tc.tile_pool(name="ps_kv", bufs=1, space="PSUM") as ps_kv,
                ):
                    rkc = rk_pool.tile([128, KVR], DT, tag="rkc", name="rkc")
                    nc.gpsimd.dma_start(rkc[:], rkc_d[:, :])
                    rks = rk_pool.tile([128, KVR], DT, tag="rks", name="rks")
                    nc.gpsimd.dma_start(rks[:], rks_d[:, :])

                    for qu in range(NTH):
                        qs = QW * qu
                        # x tiles for this third (resident xq for qu==2)
                        if qu == 2:
                            xt = xq
                        else:
                            xt = []
                            for ci in range(FD):
                                t = xkv_pool.tile([128, QW], F32R, tag="xkv",
                                                  name="xkv")
                                nc.sync.dma_start(
                                    t[:],
                                    xkv_d[128 * ci:128 * (ci + 1),
                                          qs:qs + QW])
                                xt.append(t)

                        # ---- V: ci-outer, st-inner (wv streamed per ci) ----
                        vps = [ps_kv.tile([128, NKV * D], DT, tag=f"vps{st}",
                                          name=f"vps{st}")
                               for st in range(QW // 128)]
                        for ci in range(FD):
                            wvt = wv_pool.tile([128, NKV * D], F32R, tag="wv",
                                               name="wv")
                            nc.scalar.dma_start(
                                wvt[:], wv_d[128 * ci:128 * (ci + 1), :])
                            for st in range(QW // 128):
                                nc.tensor.matmul(
                                    vps[st][:],
                                    xt[ci][:, 128 * st:128 * (st + 1)],
                                    wvt[:], start=(ci == 0),
                                    stop=(ci == FD - 1))
                        for st in range(QW // 128):
                            tl = (QW // 128) * qu + st   # kv block 0..11
                            # v data for all 8 heads in one strided copy
                            nc.scalar.copy(
                                vext[:].rearrange("p (h b w) -> p h b w",
                                                  h=NKV, b=NKB)[:, :, tl,
                                                                0:D],
                                vps[st][:].rearrange("p (h d) -> p h d",
                                                     h=NKV))
                        # validity columns for this third's blocks, all heads
                        t0 = (QW // 128) * qu
                        nc.scalar.copy(
                            vext[:].rearrange("p (h b w) -> p h b w",
                                              h=NKV, b=NKB)[
                                                  :, :, t0:t0 + QW // 128,
                                                  D:D + 1],
                            kvv[:, t0:t0 + QW // 128].rearrange(
                                "p (o b) -> p o b", o=1).to_broadcast(
                                    (128, NKV, QW // 128)))

                        # ---- K: m-outer, ci-inner (wk streamed per m,ci) ---
                        for m in range(NKV // 2):
                            kps = ps_kv.tile([128, QW], DT, tag=f"kps{m}",
                                             name=f"kps{m}")
                            for ci in range(FD):
                                wkt = wkm_pool.tile([128, 128], F32R,
                                                    tag="wkm", name="wkm")
                                nc.scalar.dma_start(
                                    wkt[:], wk_d[128 * ci:128 * (ci + 1),
                                                 128 * m:128 * (m + 1)])
                                nc.tensor.matmul(
                                    kps[:], wkt[:], xt[ci][:],
                                    start=(ci == 0), stop=(ci == FD - 1))
                            _rope_write(nc, rtmp, kT[m][:, qs:qs + QW],
                                        kps[:], rkc[:, qs:qs + QW],
                                        rks[:, qs:qs + QW], QW,
                                        swap_engine=nc.scalar)

                # ====== Q projection + attention phases 0..2 ======
                with (
                    tc.tile_pool(name="rq_tab", bufs=1) as rq_pool,
                    tc.tile_pool(name="wq_s", bufs=6) as wq_pool,
                    tc.tile_pool(name="rope_tmp_q", bufs=2) as rtmpq,
                    tc.tile_pool(name="pt", bufs=5) as pt_pool,
                    tc.tile_pool(name="att_small", bufs=2) as sm_pool,
                    tc.tile_pool(name="ps_att", bufs=1,
                                 space="PSUM") as ps_att,
                    tc.tile_pool(name="ps_q", bufs=1, space="PSUM") as ps_q,
                ):
                    rqc = rq_pool.tile([128, CH], DT, tag="rqc", name="rqc")
                    nc.gpsimd.dma_start(rqc[:], rqc_d[:, :])
                    rqs = rq_pool.tile([128, CH], DT, tag="rqs", name="rqs")
                    nc.gpsimd.dma_start(rqs[:], rqs_d[:, :])

                    def q_ci_round(sweep, ci, qps):
                        """One contraction step of q proj: 4 matmuls."""
                        wt = wq_pool.tile([128, 512], F32R, tag="wq",
                                          name="wq")
                        nc.sync.dma_start(
                            wt[:], wq_d[128 * ci:128 * (ci + 1),
                                        512 * sweep:512 * (sweep + 1)])
                        for m4 in range(4):
                            nc.tensor.matmul(qps[m4][:],
                                             wt[:, 128 * m4:128 * (m4 + 1)],
                                             xq[ci][:], start=(ci == 0),
                                             stop=(ci == FD - 1))

                    def q_rope(sweep, qps, swap_engine=None):
                        qT[sweep] = qT_pool.tile([128, REP * CH], F32R,
                                                 tag="qT", name="qT")
                        for m4 in range(4):
                            m = 4 * sweep + m4
                            boff = 64 * ((m // 2) % 2)
                            c0 = 512 * (2 * (m % 2))
                            _rope_write(
                                nc, rtmpq,
                                [qT[sweep][boff:boff + 64, c0:c0 + 512],
                                 qT[sweep][boff:boff + 64,
                                           c0 + 512:c0 + 1024]],
                                qps[m4][:], rqc[:], rqs[:], CH,
                                swap_engine=swap_engine)

                    def attention_unit(g, qt, pt_p, sm_p, ps_a):
                        """One (kv-group, q-tile) unit: 18 matmuls + exp."""
                        kTt, koff = kT[g // 2], 64 * (g % 2)
                        qv = qT[g // 2][koff:koff + 64, :].rearrange(
                            "p (r t) -> p r t", r=REP)[
                                :, :, 128 * qt:128 * (qt + 1)]
                        OT = ps_a.tile([65, REP * 128], DT, tag="OT",
                                       name="OT", bufs=2)
                        for lk in range(NWB):
                            kb = qt + lk
                            ST = ps_a.tile([128, REP * 128], DT, tag="ST",
                                           name="ST", bufs=2)
                            nc.tensor.matmul(
                                ST.rearrange("p (r t) -> p r t", r=REP),
                                kTt[koff:koff + 64, 128 * kb:128 * (kb + 1)],
                                qv, start=True, stop=True)
                            PT = pt_p.tile([128, REP * 128], F32R,
                                           tag="PT", name="PT", bufs=5)
                            nc.scalar.activation(
                                PT[:], ST[:],
                                mybir.ActivationFunctionType.Exp)
                            if lk == 0:
                                nc.vector.tensor_mul(PT[:], PT[:],
                                                     mask_win[:])
                            elif lk == NWB - 1:
                                nc.vector.tensor_mul(PT[:], PT[:],
                                                     mask_causal[:])
                            nc.tensor.matmul(
                                OT[:],
                                vext[:, VP * g + VW * kb:
                                     VP * g + VW * (kb + 1)],
                                PT[:], start=(lk == 0), stop=(lk == NWB - 1))
                        rcp = sm_p.tile([1, REP * 128], DT, tag="rcp",
                                        name="rcp")
                        nc.vector.reciprocal(rcp[:], OT[64:65, :])
                        rcpb = sm_p.tile([64, REP * 128], DT, tag="rcpb",
                                         name="rcpb")
                        nc.gpsimd.partition_broadcast(rcpb[:], rcp[:])
                        for r in range(REP):
                            h = REP * g + r
                            nc.vector.tensor_mul(
                                aT[h // 2][64 * (h % 2):64 * (h % 2) + 64,
                                           128 * qt:128 * (qt + 1)],
                                OT[0:64, 128 * r:128 * (r + 1)],
                                rcpb[:, 128 * r:128 * (r + 1)])

                    # --- q sweep 0 runs alone ---
                    qps = [ps_q.tile([128, CH], DT, tag=f"qps{m4}",
                                     name=f"qps{m4}") for m4 in range(4)]
                    for ci in range(FD):
                        q_ci_round(0, ci, qps)
                    q_rope(0, qps, swap_engine=nc.scalar)

                    # --- phases 0..2: attention pair p + q sweep p+1 ---
                    for p in range(3):
                        qps = [ps_q.tile([128, CH], DT, tag=f"qps{m4}",
                                         name=f"qps{m4}") for m4 in range(4)]
                        ci = 0
                        for g in (2 * p, 2 * p + 1):
                            for qt in range(NQT):
                                attention_unit(g, qt, pt_pool, sm_pool,
                                               ps_att)
                                for _ in range(2):
                                    if ci < FD:
                                        q_ci_round(p + 1, ci, qps)
                                        ci += 1
                        q_rope(p + 1, qps)

            # ====== phase 3 (attention pair 3 + oc0) + output proj ======
            with (
                tc.tile_pool(name="pt2", bufs=5) as pt2_pool,
                tc.tile_pool(name="att_small2", bufs=2) as sm2_pool,
                tc.tile_pool(name="wo_s", bufs=1) as wo_pool,
                tc.tile_pool(name="wo_s2", bufs=18) as wo2_pool,
                tc.tile_pool(name="ostage", bufs=4) as ostage,
                tc.tile_pool(name="ps_att2", bufs=1, space="PSUM") as ps_att2,
                tc.tile_pool(name="ps_o", bufs=1, space="PSUM") as ps_o,
            ):
                # oc0 weight tiles, prefetched on the gpsimd queue
                wot0 = [wo_pool.tile([128, 512], F32R, tag=f"wo0_{k}",
                                     name=f"wo0_{k}") for k in range(FD)]
                for k in range(FD):
                    nc.gpsimd.dma_start(
                        wot0[k][:], wo_d[128 * k:128 * (k + 1), 0:512])
                ops = [ps_o.tile([128, 512], DT, tag=f"ops{tt}",
                                 name=f"ops{tt}") for tt in range(NQT)]

                def oc0_round(k):
                    for tt in range(NQT):
                        nc.tensor.matmul(
                            ops[tt][:], aT[k][:, 128 * tt:128 * (tt + 1)],
                            wot0[k][:], start=(k == 0), stop=(k == FD - 1))

                # aT[k] is complete after unit (k//2, 3); schedule k rounds
                # 0..11 behind groups 6/7's units, 12..13 after group 6
                # finishes.
                sched = {(6, 1): [0, 1], (6, 2): [2, 3], (6, 3): [4, 5],
                         (7, 0): [6, 7, 12], (7, 1): [8, 9, 13],
                         (7, 2): [10], (7, 3): [11]}
                for g in (6, 7):
                    for qt in range(NQT):
                        attention_unit(g, qt, pt2_pool, sm2_pool, ps_att2)
                        for k in sched.get((g, qt), []):
                            oc0_round(k)
                oc0_round(14)
                oc0_round(15)
                for tt in range(NQT):
                    st = ostage.tile([128, 512], DT, tag="stage",
                                     name="stage")
                    nc.vector.tensor_copy(st[:], ops[tt][:])
                    nc.gpsimd.dma_start(
                        out_d[128 * tt:128 * (tt + 1), 0:512], st[:])

                # --- oc1..3: tt-outer / k-inner, wo per-oc resident ---
                for oc in range(1, 4):
                    wot = [wo2_pool.tile([128, 512], F32R, tag="wo2",
                                         name="wo2") for k in range(FD)]
                    for k in range(FD):
                        nc.scalar.dma_start(
                            wot[k][:], wo_d[128 * k:128 * (k + 1),
                                            512 * oc:512 * (oc + 1)])
                    for tt in range(NQT):
                        op = ps_o.tile([128, 512], DT, tag=f"ops{tt}",
                                       name=f"ops{tt}")
                        for k in range(FD):
                            nc.tensor.matmul(
                                op[:],
                                aT[k][:, 128 * tt:128 * (tt + 1)],
                                wot[k][:], start=(k == 0),
                                stop=(k == FD - 1))
                        st = ostage.tile([128, 512], DT, tag="stage",
                                         name="stage")
                        nc.vector.tensor_copy(st[:], op[:])
                        nc.gpsimd.dma_start(
                            out_d[128 * tt:128 * (tt + 1),
                                  512 * oc:512 * (oc + 1)], st[:])

    nc.compile()
    return nc


def _rope_tables(t_idx, scale):
    """cos/sin tables in [d, t] layout, 2-head packed to 128 partitions.

    Rows 0-63 and 64-127 identical; sin rows 0-31 (and 64-95) carry the
    rotate-half minus sign."""
    inv_freq = 1.0 / (ROPE_BASE ** (np.arange(0, D, 2, dtype=np.float64) / D))
    ang = t_idx[None, :] * inv_freq[:, None]          # [32, n]
    cos1 = np.cos(ang)
    sin1 = np.sin(ang)
    cos64 = np.concatenate([cos1, cos1], 0) * scale   # [64, n]
    sin64 = np.concatenate([-sin1, sin1], 0) * scale  # [64, n] signed
    return (np.tile(cos64, (2, 1)).astype(np.float32),
            np.tile(sin64, (2, 1)).astype(np.float32))


def make_in_maps(x, Wq, Wk, Wv, Wo):
    x = np.asarray(x, np.float32)
    ins = []
    i = np.arange(128)
    masks = {
        "mask_win8": np.tile((i[:, None] > i[None, :]).astype(np.float32),
                             (1, REP)),
        "mask_causal8": np.tile((i[:, None] <= i[None, :]).astype(np.float32),
                                (1, REP)),
    }
    for c in range(NCORE):
        b, ch = divmod(c, 4)
        r0 = CH * ch
        kv0 = r0 - WIN
        xT = np.ascontiguousarray(x[b].T)             # [C, T]
        xkv = np.zeros((C, KVR), np.float32)
        pad = max(0, -kv0)
        xkv[:, pad:] = xT[:, kv0 + pad:r0 + CH]
        qc, qs = _rope_tables(np.arange(r0, r0 + CH, dtype=np.float64), SCALE)
        kc, ks = _rope_tables(np.arange(kv0, r0 + CH, dtype=np.float64), 1.0)
        kvvalid = np.zeros((128, NKB), np.float32)
        for lk in range(NKB):
            kvvalid[:, lk] = (kv0 + 128 * lk + i >= 0).astype(np.float32)
        ins.append({
            "xkv": xkv,
            "wq": np.ascontiguousarray(Wq, np.float32),
            "wk": np.ascontiguousarray(Wk, np.float32),
            "wv": np.ascontiguousarray(Wv, np.float32),
            "wo": np.ascontiguousarray(Wo, np.float32),
            "rope_q_cos": qc, "rope_q_sin": qs,
            "rope_k_cos": kc, "rope_k_sin": ks,
            "kvvalid": kvvalid,
            **masks,
        })
    return ins


_PROG_CACHE = {}


def get_program():
    if "nc" not in _PROG_CACHE:
        _PROG_CACHE["nc"] = build_program()
    return _PROG_CACHE["nc"]


def kernel(x, Wq, Wk, Wv, Wo):
    nc = get_program()
    ins = make_in_maps(x, Wq, Wk, Wv, Wo)
    res = run_bass_kernel_spmd(nc, ins, list(range(NCORE)))
    out = np.empty((B, T, C), np.float32)
    for c in range(NCORE):
        b, ch = divmod(c, 4)
        out[b, CH * ch:CH * (ch + 1), :] = res.results[c]["out"]
    return out


# revision 15
# speedup vs baseline: 1.1762x; 1.1762x over previous
"""Trainium2 Bass kernel for Mistral-style sliding-window GQA attention.

Problem: B=2, T=2048, C=2048, 32 q heads / 8 kv heads, head_dim=64,
sliding causal window 1024, RoPE, fp32.

Sharding (sequence-parallel, no cross-core communication):
  core c in 0..7 handles batch b=c//4 and contiguous 512-row chunk k=c%4.
  Each core computes q for its 512 rows, k/v for its rows plus a 1024-row
  halo (zero-padded before t=0), full attention for its rows over all 32
  heads, and the output projection for its rows.  Host gathers by
  concatenation only.

Schedule (v3 — keeps the PE continuously fed):
  - KV projection per 512-col third: all V matmuls (ci-outer) first, then
    K matmuls ci-outer/m-inner with wk/wv fully resident.  Rope + vext
    copies for third t overlap the matmuls of third t+1, so the PSUM WAR
    never stalls the PE.
  - The xkv third covering the core's own 512 tokens doubles as the
    Q-projection moving operand (resident tiles, no re-DMA).
  - Q projection is emitted as 16 per-ci rounds (4 matmuls each)
    interleaved between attention units of the previous head-group pair,
    filling the PE bubbles left by the ACT-bound exp chain.
  - qT is a rotating 2-buffer: attention phase p only reads the tile
    written by q sweep p.  SBUF pool windows are arranged so kv weights,
    aT/qT, and wo streaming never coexist beyond the 208KB/partition.
  - Output projection: oc0's k-contraction rounds interleave into the
    last attention phase (PSUM banks freed by the finished q sweeps);
    oc1..3 use fresh PSUM banks (attention pools closed) so staging never
    stalls the next accumulation.
  - scores in S^T=[key, query] layout with the 4 q heads of a kv group
    packed in the moving operand (N=512, keeps float32r at 1 cyc/row);
    PV uses V_ext (64 dims + validity column) as stationary so row 64
    accumulates the softmax denominator; no max-subtraction (inputs are
    N(0,1)-scaled, exp stays in fp32 range).
"""

import numpy as np

import concourse.bass as bass
import concourse.mybir as mybir
import concourse.tile as tile
from concourse import bacc
from concourse.bass_utils import run_bass_kernel_spmd

B, T, C = 2, 2048, 2048
NH, NKV, D = 32, 8, 64
REP = NH // NKV
WIN = 1024
CH = 512          # q rows per core
KVR = CH + WIN    # kv rows per core (with halo)
NCORE = 8
DT = mybir.dt.float32
F32R = mybir.dt.float32r
SCALE = 1.0 / np.sqrt(np.float32(D))
ROPE_BASE = 10000.0

FD = T // 128     # 16 contraction tiles of the model dim
NQT = CH // 128   # 4 q tiles per chunk
NKB = KVR // 128  # 12 kv blocks per core
NWB = 9           # kv blocks in the window of one q tile
VW = 65           # v_ext width per kv block (64 dims + validity column)
VP = NKB * VW     # per-head v_ext pitch (780)


def _rope_write(nc, pool, out_ap, ps, cosw, ssinw, n, swap_engine=None):
    """out = ps*cos + rot_half(ps)*sin on a [128, n] 2-head-packed tile.

    ssinw rows carry the rotate-half signs (rows 0-31/64-95 negated) and any
    folded scale; cosw carries the same scale.  out_ap is either one [128, n]
    AP or a list of two ([64, n] AP) halves receiving rows 0:64 / 64:128.

    If swap_engine is given (an idle PSUM-capable engine, e.g. nc.scalar),
    the rotate-half shuffle is materialized there with 4 quarter copies and
    the vector engine does only 3 full-width ops; otherwise the vector
    engine does 4 quarter multiplies + 2 full ops.
    """
    if swap_engine is not None:
        sw = pool.tile([128, n], DT, tag="rope_sw", name="rope_sw")
        swap_engine.copy(sw[0:32, :], ps[32:64, :])
        swap_engine.copy(sw[32:64, :], ps[0:32, :])
        swap_engine.copy(sw[64:96, :], ps[96:128, :])
        swap_engine.copy(sw[96:128, :], ps[64:96, :])
        t2 = pool.tile([128, n], DT, tag="rope_t2", name="rope_t2")
        nc.vector.tensor_mul(t2[:], sw[:], ssinw[:])
    else:
        t2 = pool.tile([128, n], DT, tag="rope_t2", name="rope_t2")
        nc.vector.tensor_mul(t2[0:32, :], ps[32:64, :], ssinw[0:32, :])
        nc.vector.tensor_mul(t2[32:64, :], ps[0:32, :], ssinw[32:64, :])
        nc.vector.tensor_mul(t2[64:96, :], ps[96:128, :], ssinw[64:96, :])
        nc.vector.tensor_mul(t2[96:128, :], ps[64:96, :], ssinw[96:128, :])
    t1 = pool.tile([128, n], DT, tag="rope_t1", name="rope_t1")
    nc.vector.tensor_mul(t1[:], ps[:], cosw[:])
    if isinstance(out_ap, list):
        for i, half in enumerate(out_ap):
            nc.gpsimd.tensor_add(half, t1[64 * i:64 * (i + 1), :],
                                 t2[64 * i:64 * (i + 1), :])
    else:
        nc.gpsimd.tensor_add(out_ap, t1[:], t2[:])


def build_program():
    nc = bacc.Bacc("TRN2", target_bir_lowering=False, debug=False,
                   num_devices=NCORE)

    xkv_d = nc.dram_tensor("xkv", [C, KVR], F32R, kind="ExternalInput")
    wq_d = nc.dram_tensor("wq", [C, NH * D], F32R, kind="ExternalInput")
    wk_d = nc.dram_tensor("wk", [C, NKV * D], F32R, kind="ExternalInput")
    wv_d = nc.dram_tensor("wv", [C, NKV * D], F32R, kind="ExternalInput")
    wo_d = nc.dram_tensor("wo", [NH * D, C], F32R, kind="ExternalInput")
    rqc_d = nc.dram_tensor("rope_q_cos", [128, CH], DT, kind="ExternalInput")
    rqs_d = nc.dram_tensor("rope_q_sin", [128, CH], DT, kind="ExternalInput")
    rkc_d = nc.dram_tensor("rope_k_cos", [128, KVR], DT, kind="ExternalInput")
    rks_d = nc.dram_tensor("rope_k_sin", [128, KVR], DT, kind="ExternalInput")
    kvv_d = nc.dram_tensor("kvvalid", [128, NKB], F32R, kind="ExternalInput")
    mw_d = nc.dram_tensor("mask_win8", [128, 512], F32R, kind="ExternalInput")
    mc_d = nc.dram_tensor("mask_causal8", [128, 512], F32R,
                          kind="ExternalInput")
    out_d = nc.dram_tensor("out", [CH, C], DT, kind="ExternalOutput")

    NTH = 3          # x-column thirds
    QW = KVR // NTH  # 512 columns per third

    with tile.TileContext(nc) as tc:
        with (
            tc.tile_pool(name="const", bufs=1) as cpool,
            tc.tile_pool(name="kT", bufs=1) as kT_pool,
            tc.tile_pool(name="vext", bufs=1) as v_pool,
            tc.tile_pool(name="xq_res", bufs=1) as xq_pool,
        ):
            # ---- constants (small, persistent) ----
            mask_win = cpool.tile([128, 512], F32R, tag="mw", name="mask_win")
            nc.gpsimd.dma_start(mask_win[:], mw_d[:, :])
            mask_causal = cpool.tile([128, 512], F32R, tag="mc",
                                     name="mask_causal")
            nc.gpsimd.dma_start(mask_causal[:], mc_d[:, :])
            kvv = cpool.tile([128, NKB], F32R, tag="kvv", name="kvv")
            nc.gpsimd.dma_start(kvv[:], kvv_d[:, :])

            # kT: [d, t] packed 2 kv heads per tile.
            kT = [kT_pool.tile([128, KVR], F32R, tag=f"kT{i}", name=f"kT{i}")
                  for i in range(NKV // 2)]
            # vext: one tile, head kvh at pitch VP; per block 64 dims+validity
            vext = v_pool.tile([128, NKV * VP], F32R, tag="vext", name="vext")
            # resident x tiles for the core's own 512 tokens: the third
            # qu==2 of xkv, reused as the Q-projection moving operand.
            xq = [xq_pool.tile([128, CH], F32R, tag=f"xq{ci}",
                               name=f"xq{ci}") for ci in range(FD)]

            # ================= KV projection =================
            with (
                tc.tile_pool(name="rk_tab", bufs=1) as rk_pool,
                tc.tile_pool(name="wk_res", bufs=1) as wk_pool,
                tc.tile_pool(name="wv_res", bufs=1) as wv_pool,
                tc.tile_pool(name="xkv_s", bufs=17) as xkv_pool,
                tc.tile_pool(name="rope_tmp", bufs=1) as rtmp,
                tc.tile_pool(name="ps_kv", bufs=1, space="PSUM") as ps_kv,
            ):
                rkc = rk_pool.tile([128, KVR], DT, tag="rkc", name="rkc")
                nc.gpsimd.dma_start(rkc[:], rkc_d[:, :])
                rks = rk_pool.tile([128, KVR], DT, tag="rks", name="rks")
                nc.gpsimd.dma_start(rks[:], rks_d[:, :])
                # wv first (needed from the first V matmul); wk DMAs are
                # emitted after third 0's V phase so the shared DMA pipe
                # serves xkv/wv first.
                wvt = {}
                wkt = {}
                for ci in range(FD):
                    wvt[ci] = wv_pool.tile([128, NKV * D], F32R,
                                           tag=f"wv{ci}", name=f"wv{ci}")
                    nc.scalar.dma_start(
                        wvt[ci][:], wv_d[128 * ci:128 * (ci + 1), :])

                for qu in range(NTH):
                    qs = QW * qu
                    # x tiles for this third (resident xq for qu==2)
                    if qu == 2:
                        xt = xq
                    else:
                        xt = []
                        for ci in range(FD):
                            t = xkv_pool.tile([128, QW], F32R, tag="xkv",
                                              name="xkv")
                            nc.sync.dma_start(
                                t[:],
                                xkv_d[128 * ci:128 * (ci + 1), qs:qs + QW])
                            xt.append(t)

                    # ---- V: ci-outer, st-inner ----
                    vps = [ps_kv.tile([128, NKV * D], DT, tag=f"vps{st}",
                                      name=f"vps{st}")
                           for st in range(QW // 128)]
                    for ci in range(FD):
                        for st in range(QW // 128):
                            nc.tensor.matmul(
                                vps[st][:],
                                xt[ci][:, 128 * st:128 * (st + 1)],
                                wvt[ci][:], start=(ci == 0),
                                stop=(ci == FD - 1))
                    if qu == 0:
                        for ci in range(FD):
                            wkt[ci] = wk_pool.tile(
                                [128, NKV * D], F32R, tag=f"wk{ci}",
                                name=f"wk{ci}")
                            nc.scalar.dma_start(
                                wkt[ci][:],
                                wk_d[128 * ci:128 * (ci + 1), :])
                        # xq only needed from third 2 on; emitting the DMAs
                        # here keeps the early DMA pipe for xkv/wv.
                        for ci in range(FD):
                            nc.gpsimd.dma_start(
                                xq[ci][:],
                                xkv_d[128 * ci:128 * (ci + 1),
                                      WIN:WIN + CH])
                    for st in range(QW // 128):
                        tl = (QW // 128) * qu + st   # kv block 0..11
                        # v data for all 8 heads in one strided copy
                        nc.scalar.copy(
                            vext[:].rearrange("p (h b w) -> p h b w",
                                              h=NKV, b=NKB)[:, :, tl, 0:D],
                            vps[st][:].rearrange("p (h d) -> p h d", h=NKV))
                    # validity columns for this third's blocks, all heads
                    t0 = (QW // 128) * qu
                    nc.scalar.copy(
                        vext[:].rearrange("p (h b w) -> p h b w",
                                          h=NKV, b=NKB)[
                                              :, :, t0:t0 + QW // 128,
                                              D:D + 1],
                        kvv[:, t0:t0 + QW // 128].rearrange(
                            "p (o b) -> p o b", o=1).to_broadcast(
                                (128, NKV, QW // 128)))

                    # ---- K: ci-outer, m-inner; ropes at third end overlap
                    # the next third's V phase ----
                    kps = [ps_kv.tile([128, QW], DT, tag=f"kps{m}",
                                      name=f"kps{m}")
                           for m in range(NKV // 2)]
                    for ci in range(FD):
                        for m in range(NKV // 2):
                            nc.tensor.matmul(
                                kps[m][:],
                                wkt[ci][:, 128 * m:128 * (m + 1)],
                                xt[ci][:],
                                start=(ci == 0), stop=(ci == FD - 1))
                    for m in range(NKV // 2):
                        _rope_write(nc, rtmp, kT[m][:, qs:qs + QW],
                                    kps[m][:], rkc[:, qs:qs + QW],
                                    rks[:, qs:qs + QW], QW,
                                    swap_engine=nc.scalar)

            # aT/qT pools open only after the kv weight pools release.
            with (
                tc.tile_pool(name="aT", bufs=1) as aT_pool,
                tc.tile_pool(name="qT", bufs=2) as qT_pool,
            ):
                # aT: attention output, [d, t], 2 heads per tile.
                aT = [aT_pool.tile([128, CH], F32R, tag=f"aT{i}",
                                   name=f"aT{i}") for i in range(NH // 2)]
                # qT[p]: rotating per-sweep tile, rows 0:64 = group 2p (its
                # 4 heads side by side), rows 64:128 = group 2p+1.
                qT = [None] * 4

                # ====== Q projection + attention phases 0..2 ======
                with (
                    tc.tile_pool(name="rq_tab", bufs=1) as rq_pool,
                    tc.tile_pool(name="wq_s", bufs=6) as wq_pool,
                    tc.tile_pool(name="rope_tmp_q", bufs=2) as rtmpq,
                    tc.tile_pool(name="pt", bufs=5) as pt_pool,
                    tc.tile_pool(name="att_small", bufs=2) as sm_pool,
                    tc.tile_pool(name="ps_att", bufs=1,
                                 space="PSUM") as ps_att,
                    tc.tile_pool(name="ps_q", bufs=1, space="PSUM") as ps_q,
                ):
                    rqc = rq_pool.tile([128, CH], DT, tag="rqc", name="rqc")
                    nc.gpsimd.dma_start(rqc[:], rqc_d[:, :])
                    rqs = rq_pool.tile([128, CH], DT, tag="rqs", name="rqs")
                    nc.gpsimd.dma_start(rqs[:], rqs_d[:, :])

                    def q_ci_round(sweep, ci, qps):
                        """One contraction step of q proj: 4 matmuls."""
                        wt = wq_pool.tile([128, 512], F32R, tag="wq",
                                          name="wq")
                        nc.sync.dma_start(
                            wt[:], wq_d[128 * ci:128 * (ci + 1),
                                        512 * sweep:512 * (sweep + 1)])
                        for m4 in range(4):
                            nc.tensor.matmul(qps[m4][:],
                                             wt[:, 128 * m4:128 * (m4 + 1)],
                                             xq[ci][:], start=(ci == 0),
                                             stop=(ci == FD - 1))

                    def q_rope(sweep, qps, swap_engine=None):
                        qT[sweep] = qT_pool.tile([128, REP * CH], F32R,
                                                 tag="qT", name="qT")
                        for m4 in range(4):
                            m = 4 * sweep + m4
                            boff = 64 * ((m // 2) % 2)
                            c0 = 512 * (2 * (m % 2))
                            _rope_write(
                                nc, rtmpq,
                                [qT[sweep][boff:boff + 64, c0:c0 + 512],
                                 qT[sweep][boff:boff + 64,
                                           c0 + 512:c0 + 1024]],
                                qps[m4][:], rqc[:], rqs[:], CH,
                                swap_engine=swap_engine)

                    def attention_unit(g, qt, pt_p, sm_p, ps_a):
                        """One (kv-group, q-tile) unit: 18 matmuls + exp."""
                        kTt, koff = kT[g // 2], 64 * (g % 2)
                        qv = qT[g // 2][koff:koff + 64, :].rearrange(
                            "p (r t) -> p r t", r=REP)[
                                :, :, 128 * qt:128 * (qt + 1)]
                        OT = ps_a.tile([65, REP * 128], DT, tag="OT",
                                       name="OT", bufs=2)
                        for lk in range(NWB):
                            kb = qt + lk
                            ST = ps_a.tile([128, REP * 128], DT, tag="ST",
                                           name="ST", bufs=2)
                            nc.tensor.matmul(
                                ST.rearrange("p (r t) -> p r t", r=REP),
                                kTt[koff:koff + 64, 128 * kb:128 * (kb + 1)],
                                qv, start=True, stop=True)
                            PT = pt_p.tile([128, REP * 128], F32R,
                                           tag="PT", name="PT", bufs=5)
                            nc.scalar.activation(
                                PT[:], ST[:],
                                mybir.ActivationFunctionType.Exp)
                            if lk == 0:
                                nc.vector.tensor_mul(PT[:], PT[:],
                                                     mask_win[:])
                            elif lk == NWB - 1:
                                nc.vector.tensor_mul(PT[:], PT[:],
                                                     mask_causal[:])
                            nc.tensor.matmul(
                                OT[:],
                                vext[:, VP * g + VW * kb:
                                     VP * g + VW * (kb + 1)],
                                PT[:], start=(lk == 0), stop=(lk == NWB - 1))
                        rcp = sm_p.tile([1, REP * 128], DT, tag="rcp",
                                        name="rcp")
                        nc.vector.reciprocal(rcp[:], OT[64:65, :])
                        rcpb = sm_p.tile([64, REP * 128], DT, tag="rcpb",
                                         name="rcpb")
                        nc.gpsimd.partition_broadcast(rcpb[:], rcp[:])
                        for r in range(REP):
                            h = REP * g + r
                            nc.vector.tensor_mul(
                                aT[h // 2][64 * (h % 2):64 * (h % 2) + 64,
                                           128 * qt:128 * (qt + 1)],
                                OT[0:64, 128 * r:128 * (r + 1)],
                                rcpb[:, 128 * r:128 * (r + 1)])

                    # --- q sweep 0 runs alone ---
                    qps = [ps_q.tile([128, CH], DT, tag=f"qps{m4}",
                                     name=f"qps{m4}") for m4 in range(4)]
                    for ci in range(FD):
                        q_ci_round(0, ci, qps)
                    q_rope(0, qps, swap_engine=nc.scalar)

                    # --- phases 0..2: attention pair p + q sweep p+1 ---
                    for p in range(3):
                        qps = [ps_q.tile([128, CH], DT, tag=f"qps{m4}",
                                         name=f"qps{m4}") for m4 in range(4)]
                        ci = 0
                        for g in (2 * p, 2 * p + 1):
                            for qt in range(NQT):
                                attention_unit(g, qt, pt_pool, sm_pool,
                                               ps_att)
                                for _ in range(2):
                                    if ci < FD:
                                        q_ci_round(p + 1, ci, qps)
                                        ci += 1
                        q_rope(p + 1, qps)

                # ====== phase 3 (attention pair 3 + oc0) + output proj ======
                with (
                    tc.tile_pool(name="pt2", bufs=5) as pt2_pool,
                    tc.tile_pool(name="att_small2", bufs=2) as sm2_pool,
                    tc.tile_pool(name="wo_s", bufs=6) as wo_pool,
                    tc.tile_pool(name="ostage", bufs=4) as ostage,
                ):
                    def oc_round(k, oc, ops):
                        """One k-contraction round of the output projection:
                        a streamed wo tile + 4 matmuls (one per token
                        tile)."""
                        wot = wo_pool.tile([128, 512], F32R, tag="wo",
                                           name="wo")
                        nc.scalar.dma_start(
                            wot[:], wo_d[128 * k:128 * (k + 1),
                                         512 * oc:512 * (oc + 1)])
                        for tt in range(NQT):
                            nc.tensor.matmul(
                                ops[tt][:],
                                aT[k][:, 128 * tt:128 * (tt + 1)],
                                wot[:], start=(k == 0), stop=(k == FD - 1))

                    def oc_store(oc, ops):
                        for tt in range(NQT):
                            st = ostage.tile([128, 512], DT, tag="stage",
                                             name="stage")
                            nc.vector.tensor_copy(st[:], ops[tt][:])
                            nc.gpsimd.dma_start(
                                out_d[128 * tt:128 * (tt + 1),
                                      512 * oc:512 * (oc + 1)], st[:])

                    with (
                        tc.tile_pool(name="ps_att2", bufs=1,
                                     space="PSUM") as ps_att2,
                        tc.tile_pool(name="ps_o", bufs=1,
                                     space="PSUM") as ps_o,
                    ):
                        ops0 = [ps_o.tile([128, 512], DT, tag=f"ops{tt}",
                                          name=f"ops{tt}")
                                for tt in range(NQT)]
                        # aT[k] is complete after unit (k//2, 3); schedule k
                        # rounds 0..11 behind groups 6/7's units, 12..13
                        # after group 6 finishes.
                        sched = {(6, 1): [0, 1], (6, 2): [2, 3],
                                 (6, 3): [4, 5], (7, 0): [6, 7, 12],
                                 (7, 1): [8, 9, 13], (7, 2): [10],
                                 (7, 3): [11]}
                        for g in (6, 7):
                            for qt in range(NQT):
                                attention_unit(g, qt, pt2_pool, sm2_pool,
                                               ps_att2)
                                for k in sched.get((g, qt), []):
                                    oc_round(k, 0, ops0)
                        oc_round(14, 0, ops0)
                        oc_round(15, 0, ops0)
                        oc_store(0, ops0)

                    # --- oc1..3: fresh PSUM banks, k-outer streamed wo ---
                    with tc.tile_pool(name="ps_o2", bufs=2,
                                      space="PSUM") as ps_o2:
                        for oc in range(1, 4):
                            ops = [ps_o2.tile([128, 512], DT,
                                              tag=f"ops2_{tt}",
                                              name=f"ops2_{tt}")
                                   for tt in range(NQT)]
                            for k in range(FD):
                                oc_round(k, oc, ops)
                            oc_store(oc, ops)

    nc.compile()
    return nc


def _rope_tables(t_idx, scale):
    """cos/sin tables in [d, t] layout, 2-head packed to 128 partitions.

    Rows 0-63 and 64-127 identical; sin rows 0-31 (and 64-95) carry the
    rotate-half minus sign."""
    inv_freq = 1.0 / (ROPE_BASE ** (np.arange(0, D, 2, dtype=np.float64) / D))
    ang = t_idx[None, :] * inv_freq[:, None]          # [32, n]
    cos1 = np.cos(ang)
    sin1 = np.sin(ang)
    cos64 = np.concatenate([cos1, cos1], 0) * scale   # [64, n]
    sin64 = np.concatenate([-sin1, sin1], 0) * scale  # [64, n] signed
    return (np.tile(cos64, (2, 1)).astype(np.float32),
            np.tile(sin64, (2, 1)).astype(np.float32))


def make_in_maps(x, Wq, Wk, Wv, Wo):
    x = np.asarray(x, np.float32)
    ins = []
    i = np.arange(128)
    masks = {
        "mask_win8": np.tile((i[:, None] > i[None, :]).astype(np.float32),
                             (1, REP)),
        "mask_causal8": np.tile((i[:, None] <= i[None, :]).astype(np.float32),
                                (1, REP)),
    }
    for c in range(NCORE):
        b, ch = divmod(c, 4)
        r0 = CH * ch
        kv0 = r0 - WIN
        xT = np.ascontiguousarray(x[b].T)             # [C, T]
        xkv = np.zeros((C, KVR), np.float32)
        pad = max(0, -kv0)
        xkv[:, pad:] = xT[:, kv0 + pad:r0 + CH]
        qc, qs = _rope_tables(np.arange(r0, r0 + CH, dtype=np.float64), SCALE)
        kc, ks = _rope_tables(np.arange(kv0, r0 + CH, dtype=np.float64), 1.0)
        kvvalid = np.zeros((128, NKB), np.float32)
        for lk in range(NKB):
            kvvalid[:, lk] = (kv0 + 128 * lk + i >= 0).astype(np.float32)
        ins.append({
            "xkv": xkv,
            "wq": np.ascontiguousarray(Wq, np.float32),
            "wk": np.ascontiguousarray(Wk, np.float32),
            "wv": np.ascontiguousarray(Wv, np.float32),
            "wo": np.ascontiguousarray(Wo, np.float32),
            "rope_q_cos": qc, "rope_q_sin": qs,
            "rope_k_cos": kc, "rope_k_sin": ks,
            "kvvalid": kvvalid,
            **masks,
        })
    return ins


_PROG_CACHE = {}


def get_program():
    if "nc" not in _PROG_CACHE:
        _PROG_CACHE["nc"] = build_program()
    return _PROG_CACHE["nc"]


def kernel(x, Wq, Wk, Wv, Wo):
    nc = get_program()
    ins = make_in_maps(x, Wq, Wk, Wv, Wo)
    res = run_bass_kernel_spmd(nc, ins, list(range(NCORE)))
    out = np.empty((B, T, C), np.float32)
    for c in range(NCORE):
        b, ch = divmod(c, 4)
        out[b, CH * ch:CH * (ch + 1), :] = res.results[c]["out"]
    return out
